# revision 42
# baseline (speedup 1.0000x reference)
"""DecoderRNN Trainium2 kernel: 63-step LSTM + Luong attention + vocab projection.

Strategy (8 NeuronCores, SPMD), v3 — pipelined single pass, split-bf16 h:
  - Recurrence TP-8: each core owns 128 hidden dims x 4 gates (chunk order
    i,f,o,g). Gate psum [128, 4, 32] accumulates bias (one-hot matmul) +
    W_ih x_t (4 k-tiles/chunk) + W_hh h_{t-1} (8 k-tiles/chunk), all bf16
    (FWL weight loads, 1 cyc/col streams).
  - h is carried as a bf16 (hi, lo) pair -- hi = bf16(h), lo = bf16(h - hi)
    -- restoring ~fp23 accuracy in the h @ W_hh and h @ encT products while
    keeping full-rate bf16 matmuls. Per-step AllGather moves the pair
    ([128, 64] bf16); one DMA scatters cc_out into the hall ring.
  - Attention/decoder/vocab work for a finished block of steps is emitted
    as small "filler" quanta between later recurrence steps, so the PE
    chews on it during each AllGather's ~5-6us latency (also keeps the
    HAM clock-gate warm). Softmax uses exp(x)=(1+t)/(1-t), t=tanh(x/2)
    (x<=max-shifted<=0, numerically safe) so the whole kernel stays on the
    sigmoid/tanh ACT table set -- no 2.7us table reloads mid-pipeline.
  - Block cols are b-major (b,t); vocab m-tiles are fixed 128-col windows
    (FWL) and output DMAs split at batch boundaries. Logits emitted bf16;
    host casts to f32. Vocab V-sharded 8 ways; host concatenates.
"""

import numpy as np
import ml_dtypes
from contextlib import ExitStack

import concourse.bass as bass
import concourse.bacc as bacc
import concourse.tile as tile
import concourse.mybir as mybir
from concourse import masks
from concourse.bass_utils import run_bass_kernel_spmd

F32 = mybir.dt.float32
BF16 = mybir.dt.bfloat16
FP16 = mybir.dt.float16
AF = mybir.ActivationFunctionType
ALU = mybir.AluOpType
AX = mybir.AxisListType

B, T, S = 32, 63, 64          # batch, steps (T-1 of the 64), source len
V, E, H = 32000, 512, 1024
P = 128                       # partitions
NCORES = 8
R = T * B                     # 2016 rows, recurrence col index r = t*32 + b
VL = V // NCORES              # 4000 vocab cols per core
KH = H // P                   # 8 k-chunks over hidden
KE = E // P                   # 4 k-chunks over embedding
NQ = 4                        # gate chunks owned per core (i, f, o, g)
RING = 24                     # hall ring slots
VN = 8                        # vocab n-tiles of 500
VT = VL // VN                 # 500
# decreasing block sizes: block j's attention/dec/vocab filler drains during
# block j+1's AllGather gaps; later blocks shrink to limit the exposed tail.
BLOCK_SIZES = [12, 12, 12, 12, 8, 4, 3]
assert sum(BLOCK_SIZES) == T
BLOCKS = []
_t0 = 0
for _bs in BLOCK_SIZES:
    BLOCKS.append((_t0, _t0 + _bs))
    _t0 += _bs
DEBUG = True


def _dma_segments(m0, mw, bs):
    """Split dect col window [m0, m0+mw) at batch boundaries.

    Returns (b_start, t_off, t_take, rel_row, n_batches) with full-batch
    runs merged, so each segment is a rectangle in (b, t)."""
    raw = []
    r = m0
    while r < m0 + mw:
        b, off = divmod(r, bs)
        take = min(m0 + mw - r, bs - off)
        raw.append((b, off, take, r - m0))
        r += take
    merged = []
    for b, off, take, rel in raw:
        if (merged and off == 0 and take == bs and merged[-1][1] == 0
                and merged[-1][2] == bs and merged[-1][0] + merged[-1][4] == b):
            merged[-1] = merged[-1][:4] + (merged[-1][4] + 1,)
            continue
        merged.append((b, off, take, rel, 1))
    return merged


def build_graph():
    nc = bacc.Bacc("TRN2", target_bir_lowering=False, debug=False,
                   num_devices=NCORES)

    def inp(name, shape, dtype):
        return nc.dram_tensor(name, list(shape), dtype, kind="ExternalInput").ap()

    x_embT = inp("x_embT", [E, R], FP16)            # embedded tgt, (k p) x (t,b)
    w_ihT_s = inp("w_ihT_s", [E, NQ * P], FP16)     # cols (c,p), c in (i,f,o,g)
    w_hhT_s = inp("w_hhT_s", [H, NQ * P], FP16)
    bias_s = inp("bias_s", [P, NQ], F32)            # (b_ih+b_hh) per owned chunk
    h0T = inp("h0T", [H, B], FP16)              # h0 split pair (hi, lo)
    c0T_s = inp("c0T_s", [P, B], F32)               # own hidden chunk of c0
    enc = inp("enc", [B, S, H], FP16)               # ctx lhsT
    encT = inp("encT", [B, H, S], FP16)             # scores rhs
    w_wT = inp("w_wT", [2 * H, H], FP16)
    b_w_sb = inp("b_w_sb", [P, KH], F32)
    w_outT_s = inp("w_outT_s", [H, VL], FP16)       # per-core vocab slice
    b_out_s = inp("b_out_s", [1, VL], FP16)
    out_s = nc.dram_tensor("out_s", [B, T, VL], FP16, kind="ExternalOutput").ap()
    if DEBUG:
        dbg_h = nc.dram_tensor("dbg_h", [T, P, 2 * B], BF16,
                               kind="ExternalOutput").ap()
        dbg_pn = nc.dram_tensor("dbg_pn", [len(BLOCKS), 16, B * S], BF16,
                                kind="ExternalOutput").ap()
        dbg_ctx = nc.dram_tensor("dbg_ctx", [P, KH, R], BF16,
                                 kind="ExternalOutput").ap()
        dbg_dec = nc.dram_tensor("dbg_dec", [P, KH, R], BF16,
                                 kind="ExternalOutput").ap()
        dbg_g = nc.dram_tensor("dbg_g", [2, P, 6 * NQ * B], F32,
                               kind="ExternalOutput").ap()

    x_embT_r = x_embT.rearrange("(k p) r -> p k r", p=P)

    with tile.TileContext(nc) as tc, ExitStack() as ctx:
        pool1 = ctx.enter_context(tc.tile_pool(name="pool1", bufs=1))
        stream = ctx.enter_context(tc.tile_pool(name="stream", bufs=2))
        work = ctx.enter_context(tc.tile_pool(name="work", bufs=2))
        state = ctx.enter_context(tc.tile_pool(name="state", bufs=2))
        psum = ctx.enter_context(tc.tile_pool(name="psum", bufs=2, space="PSUM"))
        dram = ctx.enter_context(tc.tile_pool(name="dram", bufs=1, space="DRAM"))

        # ---------------- resident tiles (small/critical first) ----------------
        whh = pool1.tile([P, KH, NQ * P], FP16, name="whh")
        nc.sync.dma_start(whh[:], w_hhT_s.rearrange("(k p) c -> p k c", p=P))
        wih = pool1.tile([P, KE, NQ * P], FP16, name="wih")
        nc.sync.dma_start(wih[:], w_ihT_s.rearrange("(k p) c -> p k c", p=P))
        bias_t = pool1.tile([P, NQ], F32, name="bias_t")
        nc.sync.dma_start(bias_t[:], bias_s[:])
        bw_t = pool1.tile([P, KH], F32, name="bw_t")
        nc.sync.dma_start(bw_t[:], b_w_sb[:])
        bout_t = pool1.tile([1, VL], FP16, name="bout_t")
        nc.sync.dma_start(bout_t[:], b_out_s[:])
        ones_t = pool1.tile([1, P], FP16, name="ones_t")
        nc.gpsimd.memset(ones_t[:], 1.0)
        h0_t = pool1.tile([P, KH, B], FP16, name="h0_t")
        nc.sync.dma_start(h0_t[:], h0T.rearrange("(k p) b -> p k b", p=P))
        c0_sb = pool1.tile([P, B], F32, name="c0_sb")
        nc.sync.dma_start(c0_sb[:], c0T_s[:])
        ident = pool1.tile([P, P], FP16, name="ident")
        masks.make_identity(nc, ident[:])
        # big weights on the scalar queue so they don't delay the first steps
        ww = pool1.tile([P, 2 * KH, H], FP16, name="ww")
        nc.scalar.dma_start(ww[:], w_wT.rearrange("(k p) m -> p k m", p=P))
        wout = pool1.tile([P, KH, VL], FP16, name="wout")
        nc.scalar.dma_start(wout[:], w_outT_s.rearrange("(k p) v -> p k v", p=P))

        # hall ring: h (fp16) for the last RING steps
        hall = pool1.tile([P, KH, RING, B], FP16, name="hall")

        cc_in = [dram.tile([P, B], FP16, name=f"cc_in{i}") for i in range(T)]
        cc_out = [dram.tile([NCORES * P, B], FP16, name=f"cc_out{i}",
                            addr_space="Shared") for i in range(T)]

        # per-block x_emb tiles, prefetched one block ahead
        xe_tiles = {}
        xg_tiles = {}

        def fetch_xe(bi):
            t0, t1 = BLOCKS[bi]
            xe = stream.tile([P, KE, B * (t1 - t0)], FP16, name="xe",
                             tag=f"xe{bi % 2}", bufs=1)
            nc.sync.dma_start(xe[:], x_embT_r[:, :, t0 * B:t1 * B])
            xe_tiles[bi] = xe

        # Xg = W_ih x + bias for a whole block, emitted as an early filler
        # closure one block ahead of use
        def mk_xg(bi):
            t0, t1 = BLOCKS[bi]
            cols = B * (t1 - t0)
            xg = work.tile([P, NQ, 512], F32, name="xg", tag="xg", bufs=2)
            xg_tiles[bi] = xg
            xe = xe_tiles[bi]

            def emit():
                for c in range(NQ):
                    ps_x = psum.tile([P, 512], F32, name="ps_x", tag="ps_dec",
                                     bufs=1)
                    for k in range(KE):
                        nc.tensor.matmul(
                            ps_x[:, :cols],
                            lhsT=wih[:, k, c * P:(c + 1) * P],
                            rhs=xe[:, k, :],
                            start=(k == 0), stop=(k == KE - 1))
                    nc.scalar.activation(xg[:, c, :cols], ps_x[:, :cols],
                                         AF.Identity, bias=bias_t[:, c:c + 1])
            return emit

        # ---------------- filler emission (attention/dec/vocab per block) ----
        def block_closures(bi):
            t0, t1 = BLOCKS[bi]
            bs = t1 - t0
            cols = B * bs            # block cols, b-major (b, t)
            r0 = t0 % RING
            cls = []

            # per-block tiles (tag-rotated, 2 bufs -> adjacent blocks overlap)
            pn_t = work.tile([16, B, S], FP16, name="pn", tag="pn_blk", bufs=1)
            at_t = work.tile([S, B, 16], FP16, name="at", tag="at_blk", bufs=1)
            ctxb = work.tile([P, KH, cols], FP16, name="ctxb", tag="ctx_blk", bufs=1)
            decb = work.tile([P, KH, cols], FP16, name="decb", tag="dec_blk", bufs=1)

            # --- scores + softmax, one closure per quad of 4 batches ---
            def mk_scores(q):
                def emit():
                    ps_s = psum.tile([16, 4, S], F32, name="ps_s", tag="ps_sc",
                                     bufs=1)
                    et4 = stream.tile([P, 4, KH, S], FP16, name="et4",
                                      tag="et4", bufs=2)
                    nc.sync.dma_start(
                        et4[:], encT[4 * q:4 * q + 4, :, :].rearrange(
                            "b (k p) s -> p b k s", p=P))
                    for bq in range(4):
                        b = q * 4 + bq
                        for k in range(KH):
                            nc.tensor.matmul(
                                ps_s[:bs, bq, :],
                                lhsT=hall[:, k, r0:r0 + bs, b],
                                rhs=et4[:, bq, k, :],
                                start=(k == 0), stop=(k == KH - 1))
                    # softmax over s via exp(x) = (1+t)/(1-t), t = tanh(x/2)
                    mxn = work.tile([16, 1], F32, name="mxn", tag="mxn")
                    nc.vector.tensor_reduce(mxn[:bs, :], ps_s[:bs, :, :],
                                            axis=AX.XY, op=ALU.max, negate=True)
                    nmx2 = work.tile([16, 1], F32, name="nmx2", tag="nmx2")
                    nc.vector.tensor_scalar_mul(nmx2[:bs, :], mxn[:bs, :], 0.5)
                    tq = work.tile([16, 4, S], F32, name="tq", tag="tq", bufs=1)
                    nc.scalar.activation(tq[:bs, :, :], ps_s[:bs, :, :],
                                         AF.Tanh, bias=nmx2[:bs, :], scale=0.5)
                    un = work.tile([16, 4, S], F32, name="un", tag="un", bufs=1)
                    nc.vector.tensor_scalar_add(un[:bs, :, :], tq[:bs, :, :], 1.0)
                    dn = work.tile([16, 4, S], F32, name="dn", tag="dn", bufs=1)
                    nc.vector.tensor_scalar(dn[:bs, :, :], tq[:bs, :, :],
                                            -1.0, 1.0, ALU.mult, ALU.add)
                    vr = work.tile([16, 4, S], F32, name="vr", tag="vr", bufs=1)
                    nc.vector.reciprocal(vr[:bs, :, :], dn[:bs, :, :])
                    pu = work.tile([16, 4, S], F32, name="pu", tag="pu", bufs=1)
                    nc.vector.tensor_tensor(out=pu[:bs, :, :], in0=un[:bs, :, :],
                                            in1=vr[:bs, :, :], op=ALU.mult)
                    zs = work.tile([16, 4], F32, name="zs", tag="zs")
                    nc.vector.tensor_reduce(zs[:bs, :], pu[:bs, :, :],
                                            axis=AX.X, op=ALU.add)
                    rz = work.tile([16, 4], F32, name="rz", tag="rz")
                    nc.vector.reciprocal(rz[:bs, :], zs[:bs, :])
                    for bq in range(4):
                        b = q * 4 + bq
                        nc.vector.tensor_scalar_mul(
                            pn_t[:bs, b, :], pu[:bs, bq, :],
                            rz[:bs, bq:bq + 1])
                return emit

            # pn_t holds probs [t, b, s]; transpose each b to [s, t]
            def mk_transp(pg):
                def emit():
                    if DEBUG and pg == 0:
                        nc.sync.dma_start(
                            dbg_pn[bi, :, :],
                            pn_t[:].rearrange("t b s -> t (b s)"))
                    for b in range(4 * pg, 4 * pg + 4):
                        ps_t = psum.tile([S, 16], FP16, name="ps_t",
                                         tag="ps_tr", bufs=1)
                        nc.tensor.transpose(
                            ps_t[:, :bs], pn_t[:bs, b, :], ident[:bs, :bs])
                        nc.vector.tensor_copy(at_t[:, b, :bs], ps_t[:, :bs])
                return emit

            # context: per k-chunk, all 32 b into one psum bank, b-major cols
            def mk_ctx(k):
                def emit():
                    eca = stream.tile([S, B, P], FP16, name="eca", tag="eca",
                                      bufs=1)
                    nc.sync.dma_start(
                        eca[:], enc[:, :, k * P:(k + 1) * P].rearrange(
                            "b s h -> s b h"))
                    ps_c = psum.tile([P, 512], F32, name="ps_c", tag="ps_ctx",
                                     bufs=1)
                    for b in range(B):
                        nc.tensor.matmul(
                            ps_c[:, b * bs:(b + 1) * bs],
                            lhsT=eca[:, b, :],
                            rhs=at_t[:, b, :bs],
                            start=True, stop=True)
                    nc.vector.tensor_copy(ctxb[:, k, :], ps_c[:, :cols])
                    if DEBUG:
                        nc.sync.dma_start(dbg_ctx[:, k, t0 * B:t1 * B],
                                          ctxb[:, k, :])
                return emit

            # dec = tanh(W_w^T [h; ctx] + b_w), per output chunk mo
            def mk_dec(mo):
                def emit():
                    ps_d = psum.tile([P, 512], F32, name="ps_d", tag="ps_dec",
                                     bufs=1)
                    for k in range(KH):
                        nc.tensor.matmul(
                            ps_d[:, :cols],
                            lhsT=ww[:, k, mo * P:(mo + 1) * P],
                            rhs=hall[:, k, r0:r0 + bs, :].rearrange(
                                "p t b -> p b t"),
                            start=(k == 0), stop=False)
                    for k in range(KH):
                        nc.tensor.matmul(
                            ps_d[:, :cols],
                            lhsT=ww[:, KH + k, mo * P:(mo + 1) * P],
                            rhs=ctxb[:, k, :],
                            start=False, stop=(k == KH - 1))
                    nc.scalar.activation(decb[:, mo, :], ps_d[:, :cols],
                                         AF.Tanh, bias=bw_t[:, mo:mo + 1])
                    if DEBUG:
                        nc.sync.dma_start(dbg_dec[:, mo, t0 * B:t1 * B],
                                          decb[:, mo, :])
                return emit

            # vocab: per n-tile of 500; m-tiles are 128-col windows (FWL)
            def mk_vocab(n):
                def emit():
                    nm = -(-cols // P)
                    for g in range(nm):
                        m0 = g * P
                        mw = min(P, cols - m0)
                        ps_v = psum.tile([P, VT], F32, name="ps_v", tag="ps_v")
                        for k in range(KH):
                            nc.tensor.matmul(
                                ps_v[:mw, :],
                                lhsT=decb[:, k, m0:m0 + mw],
                                rhs=wout[:, k, n * VT:(n + 1) * VT],
                                start=(k == 0), stop=(k == KH - 1))
                        o_sb = work.tile([P, VT], FP16, name="o_sb", tag="o_sb")
                        nc.vector.tensor_copy(o_sb[:mw, :], ps_v[:mw, :])
                        for (b0, toff, ttake, rel, nb) in _dma_segments(
                                m0, mw, bs):
                            nc.scalar.dma_start(
                                out_s[b0:b0 + nb, t0 + toff:t0 + toff + ttake,
                                      n * VT:(n + 1) * VT],
                                o_sb[rel:rel + nb * ttake, :])
                return emit

            for q in range(8):
                cls.append(mk_scores(q))
            for pg in range(8):
                cls.append(mk_transp(pg))
            for k in range(KH):
                cls.append(mk_ctx(k))
            for mo in range(KH):
                cls.append(mk_dec(mo))
            for n in range(VN):
                cls.append(mk_vocab(n))
            return cls

        # ---------------- recurrence with interleaved filler ----------------
        fetch_xe(0)
        fetch_xe(1)
        mk_xg(0)()
        mk_xg(1)()
        pending = []
        blocks_done = 0
        cur_blk = 0
        c_prev = c0_sb
        for t in range(T):
            if t >= BLOCKS[cur_blk][1]:
                cur_blk += 1
                if cur_blk + 1 < len(BLOCKS):
                    fetch_xe(cur_blk + 1)
                    pending.insert(0, mk_xg(cur_blk + 1))
            t0b = BLOCKS[cur_blk][0]
            xg = xg_tiles[cur_blk]
            rt = t % RING
            # gates psum [p, (c, b)], c in (i, f, o, g)
            # NB: start=True zeroes the whole 2KB psum bank (pending-zero is
            # bank-granular), so exactly ONE start per step.
            psg = psum.tile([P, NQ, B], F32, name="psg", tag="psg", bufs=1)
            # W_hh h_{t-1}: stalls until AllGather(t-1) has landed
            for c in range(NQ):
                for k in range(KH):
                    rhs = (h0_t[:, k, :] if t == 0
                           else hall[:, k, (t - 1) % RING, :])
                    nc.tensor.matmul(
                        psg[:, c, :],
                        lhsT=whh[:, k, c * P:(c + 1) * P],
                        rhs=rhs,
                        start=(c == 0 and k == 0),
                        stop=(c == NQ - 1 and k == KH - 1))
            # gates = psum + Xg, landed in PSUM so the ACT reads dodge the
            # SBUF-source errata; sigmoid(i,f,o) one instr, tanh(g)
            gs = psum.tile([P, NQ, B], F32, name="gs", tag="gs", bufs=1)
            nc.vector.tensor_tensor(
                out=gs[:], in0=psg[:],
                in1=xg[:, :, (t - t0b) * B:(t - t0b + 1) * B],
                op=ALU.add)
            sfo = work.tile([P, 3, B], F32, name="sfo", tag="sfo")
            nc.scalar.activation(sfo[:], gs[:, 0:3, :], AF.Sigmoid)
            tg = work.tile([P, B], F32, name="tg", tag="tg")
            nc.scalar.activation(tg[:], gs[:, 3, :], AF.Tanh)
            t1_ = work.tile([P, B], F32, name="t1_", tag="t1_")
            nc.vector.tensor_tensor(out=t1_[:], in0=sfo[:, 1, :],
                                    in1=c_prev[:], op=ALU.mult)
            t2_ = work.tile([P, B], F32, name="t2_", tag="t2_")
            nc.vector.tensor_tensor(out=t2_[:], in0=sfo[:, 0, :], in1=tg[:],
                                    op=ALU.mult)
            c_new = state.tile([P, B], F32, name="c_new", tag="c_new")
            nc.vector.tensor_tensor(out=c_new[:], in0=t1_[:], in1=t2_[:],
                                    op=ALU.add)
            c_prev = c_new
            tc_t = work.tile([P, B], F32, name="tc_t", tag="tc_t")
            nc.scalar.activation(tc_t[:], c_new[:], AF.Tanh)
            h_f = work.tile([P, B], FP16, name="h_f", tag="h_f")
            nc.vector.tensor_tensor(out=h_f[:], in0=sfo[:, 2, :],
                                    in1=tc_t[:], op=ALU.mult)
            # exchange h pair: SBUF -> DRAM -> AllGather -> hall ring
            nc.gpsimd.dma_start(cc_in[t][:], h_f[:])
            nc.gpsimd.collective_compute(
                "AllGather", ALU.bypass,
                replica_groups=[list(range(NCORES))],
                ins=[cc_in[t].opt()],
                outs=[cc_out[t].opt()])
            cc_r = cc_out[t].rearrange("(k p) b -> p k b", p=P)
            for k in range(KH):
                eng = (nc.sync, nc.scalar, nc.gpsimd)[k % 3]
                eng.dma_start(hall[:, k, rt, :], cc_r[:, k, :])
            # drain filler into this step's AG gap; pace so each block's
            # closures spread over the steps before the next batch arrives
            if pending:
                nxt = BLOCKS[blocks_done][1] if blocks_done < len(BLOCKS) else T
                quota = -(-len(pending) // max(1, (3 * (nxt - t)) // 4))
                for _ in range(quota):
                    if pending:
                        pending.pop(0)()
            if blocks_done < len(BLOCKS) and t + 1 == BLOCKS[blocks_done][1]:
                pending.extend(block_closures(blocks_done))
                blocks_done += 1
        while pending:
            pending.pop(0)()
    nc.compile()
    return nc


_CACHE = {}


def _get_graph():
    if "nc" not in _CACHE:
        _CACHE["nc"] = build_graph()
    return _CACHE["nc"]


def _prep(tgt_input, hidden_state, cell_state, encoder_outputs,
          embedding, W_ih, W_hh, b_ih, b_hh, W_w, b_w, W_out, b_out):
    """Host-side layout prep. Returns per-core input maps."""
    f32 = np.float32
    bf16 = np.float16
    idx = np.asarray(tgt_input)[:, :-1].astype(np.int64)    # [B, T]
    emb = np.asarray(embedding, f32)[idx]                   # [B, T, E]
    x_embT = np.ascontiguousarray(
        emb.transpose(2, 1, 0).reshape(E, R)).astype(bf16)

    w_ihT = np.asarray(W_ih, f32).T                         # [E, 4H]
    w_hhT = np.asarray(W_hh, f32).T                         # [H, 4H]
    bias = (np.asarray(b_ih, f32) + np.asarray(b_hh, f32))  # [4H]
    h0T_a = np.ascontiguousarray(
        np.asarray(hidden_state, f32)[0].T).astype(bf16)    # [H, B]
    c0T = np.ascontiguousarray(np.asarray(cell_state, f32)[0].T)  # [H, B]
    enc_b = np.asarray(encoder_outputs, f32).astype(bf16)   # [B, S, H]
    encT_b = np.ascontiguousarray(
        np.asarray(encoder_outputs, f32).transpose(0, 2, 1)).astype(bf16)
    w_wT_full = np.ascontiguousarray(np.asarray(W_w, f32).T).astype(bf16)
    b_w_sb = np.ascontiguousarray(np.asarray(b_w, f32).reshape(KH, P).T)
    w_outT = np.asarray(W_out, f32).T                       # [H, V]
    b_out_a = np.asarray(b_out, f32)

    in_maps = []
    for m in range(NCORES):
        # owned gate cols, chunk order (i, f, o, g); PyTorch gate order
        # along 4H is (i, f, g, o) -> quarters (0, 1, 3, 2)
        cols = np.concatenate([np.arange(q * H + m * P, q * H + (m + 1) * P)
                               for q in (0, 1, 3, 2)])
        in_maps.append({
            "x_embT": x_embT,
            "w_ihT_s": np.ascontiguousarray(w_ihT[:, cols]).astype(bf16),
            "w_hhT_s": np.ascontiguousarray(w_hhT[:, cols]).astype(bf16),
            "bias_s": np.ascontiguousarray(bias[cols].reshape(NQ, P).T),
            "h0T": h0T_a,
            "c0T_s": np.ascontiguousarray(c0T[m * P:(m + 1) * P, :]),
            "enc": enc_b,
            "encT": encT_b,
            "w_wT": w_wT_full,
            "b_w_sb": b_w_sb,
            "w_outT_s": np.ascontiguousarray(
                w_outT[:, m * VL:(m + 1) * VL]).astype(bf16),
            "b_out_s": np.ascontiguousarray(
                b_out_a[m * VL:(m + 1) * VL]).reshape(1, VL).astype(bf16),
        })
    return in_maps


def kernel(**inputs) -> np.ndarray:
    nc = _get_graph()
    in_maps = _prep(**inputs)
    res = run_bass_kernel_spmd(nc, in_maps, list(range(NCORES)))
    outs = [np.asarray(res.results[m]["out_s"], dtype=np.float32)
            for m in range(NCORES)]
    full = np.concatenate(outs, axis=2)
    b_out = np.asarray(inputs["b_out"], np.float32)
    if np.any(b_out):
        full = full + b_out
    return full


# revision 44
# speedup vs baseline: 1.0305x; 1.0305x over previous
"""DecoderRNN Trainium2 kernel: 63-step LSTM + Luong attention + vocab projection.

Strategy (8 NeuronCores, SPMD), v3 — pipelined single pass, split-bf16 h:
  - Recurrence TP-8: each core owns 128 hidden dims x 4 gates (chunk order
    i,f,o,g). Gate psum [128, 4, 32] accumulates bias (one-hot matmul) +
    W_ih x_t (4 k-tiles/chunk) + W_hh h_{t-1} (8 k-tiles/chunk), all bf16
    (FWL weight loads, 1 cyc/col streams).
  - h is carried as a bf16 (hi, lo) pair -- hi = bf16(h), lo = bf16(h - hi)
    -- restoring ~fp23 accuracy in the h @ W_hh and h @ encT products while
    keeping full-rate bf16 matmuls. Per-step AllGather moves the pair
    ([128, 64] bf16); one DMA scatters cc_out into the hall ring.
  - Attention/decoder/vocab work for a finished block of steps is emitted
    as small "filler" quanta between later recurrence steps, so the PE
    chews on it during each AllGather's ~5-6us latency (also keeps the
    HAM clock-gate warm). Softmax uses exp(x)=(1+t)/(1-t), t=tanh(x/2)
    (x<=max-shifted<=0, numerically safe) so the whole kernel stays on the
    sigmoid/tanh ACT table set -- no 2.7us table reloads mid-pipeline.
  - Block cols are b-major (b,t); vocab m-tiles are fixed 128-col windows
    (FWL) and output DMAs split at batch boundaries. Logits emitted bf16;
    host casts to f32. Vocab V-sharded 8 ways; host concatenates.
"""

import numpy as np
import ml_dtypes
from contextlib import ExitStack

import concourse.bass as bass
import concourse.bacc as bacc
import concourse.tile as tile
import concourse.mybir as mybir
from concourse import masks
from concourse.bass_utils import run_bass_kernel_spmd

F32 = mybir.dt.float32
BF16 = mybir.dt.bfloat16
FP16 = mybir.dt.float16
AF = mybir.ActivationFunctionType
ALU = mybir.AluOpType
AX = mybir.AxisListType

B, T, S = 32, 63, 64          # batch, steps (T-1 of the 64), source len
V, E, H = 32000, 512, 1024
P = 128                       # partitions
NCORES = 8
R = T * B                     # 2016 rows, recurrence col index r = t*32 + b
VL = V // NCORES              # 4000 vocab cols per core
KH = H // P                   # 8 k-chunks over hidden
KE = E // P                   # 4 k-chunks over embedding
NQ = 4                        # gate chunks owned per core (i, f, o, g)
RING = 24                     # hall ring slots
VN = 8                        # vocab n-tiles of 500
VT = VL // VN                 # 500
# decreasing block sizes: block j's attention/dec/vocab filler drains during
# block j+1's AllGather gaps; later blocks shrink to limit the exposed tail.
BLOCK_SIZES = [12, 12, 12, 12, 8, 4, 3]
assert sum(BLOCK_SIZES) == T
BLOCKS = []
_t0 = 0
for _bs in BLOCK_SIZES:
    BLOCKS.append((_t0, _t0 + _bs))
    _t0 += _bs
DEBUG = True


def _dma_segments(m0, mw, bs):
    """Split dect col window [m0, m0+mw) at batch boundaries.

    Returns (b_start, t_off, t_take, rel_row, n_batches) with full-batch
    runs merged, so each segment is a rectangle in (b, t)."""
    raw = []
    r = m0
    while r < m0 + mw:
        b, off = divmod(r, bs)
        take = min(m0 + mw - r, bs - off)
        raw.append((b, off, take, r - m0))
        r += take
    merged = []
    for b, off, take, rel in raw:
        if (merged and off == 0 and take == bs and merged[-1][1] == 0
                and merged[-1][2] == bs and merged[-1][0] + merged[-1][4] == b):
            merged[-1] = merged[-1][:4] + (merged[-1][4] + 1,)
            continue
        merged.append((b, off, take, rel, 1))
    return merged


def build_graph():
    nc = bacc.Bacc("TRN2", target_bir_lowering=False, debug=False,
                   num_devices=NCORES)

    def inp(name, shape, dtype):
        return nc.dram_tensor(name, list(shape), dtype, kind="ExternalInput").ap()

    x_embT = inp("x_embT", [E, R], FP16)            # embedded tgt, (k p) x (t,b)
    w_ihT_s = inp("w_ihT_s", [E, NQ * P], FP16)     # cols (c,p), c in (i,f,o,g)
    w_hhT_s = inp("w_hhT_s", [H, NQ * P], FP16)
    biasT_s = inp("biasT_s", [NQ, P], FP16)         # (b_ih+b_hh)[c, p]
    onehot_s = inp("onehot_s", [NQ, NQ * B], FP16)  # [k, (c,b)] = (k==c)
    h0T = inp("h0T", [H, B], FP16)              # h0 split pair (hi, lo)
    c0T_s = inp("c0T_s", [P, B], F32)               # own hidden chunk of c0
    enc = inp("enc", [B, S, H], FP16)               # ctx lhsT
    encT = inp("encT", [B, H, S], FP16)             # scores rhs
    w_wT = inp("w_wT", [2 * H, H], FP16)
    b_w_sb = inp("b_w_sb", [P, KH], F32)
    w_outT_s = inp("w_outT_s", [H, VL], FP16)       # per-core vocab slice
    b_out_s = inp("b_out_s", [1, VL], FP16)
    out_s = nc.dram_tensor("out_s", [B, T, VL], FP16, kind="ExternalOutput").ap()
    if DEBUG:
        dbg_h = nc.dram_tensor("dbg_h", [T, P, 2 * B], BF16,
                               kind="ExternalOutput").ap()
        dbg_pn = nc.dram_tensor("dbg_pn", [len(BLOCKS), 16, B * S], BF16,
                                kind="ExternalOutput").ap()
        dbg_ctx = nc.dram_tensor("dbg_ctx", [P, KH, R], BF16,
                                 kind="ExternalOutput").ap()
        dbg_dec = nc.dram_tensor("dbg_dec", [P, KH, R], BF16,
                                 kind="ExternalOutput").ap()
        dbg_g = nc.dram_tensor("dbg_g", [2, P, 6 * NQ * B], F32,
                               kind="ExternalOutput").ap()

    x_embT_r = x_embT.rearrange("(k p) r -> p k r", p=P)

    with tile.TileContext(nc) as tc, ExitStack() as ctx:
        pool1 = ctx.enter_context(tc.tile_pool(name="pool1", bufs=1))
        stream = ctx.enter_context(tc.tile_pool(name="stream", bufs=2))
        work = ctx.enter_context(tc.tile_pool(name="work", bufs=2))
        state = ctx.enter_context(tc.tile_pool(name="state", bufs=2))
        psum = ctx.enter_context(tc.tile_pool(name="psum", bufs=2, space="PSUM"))
        dram = ctx.enter_context(tc.tile_pool(name="dram", bufs=1, space="DRAM"))

        # ---------------- resident tiles (small/critical first) ----------------
        whh = pool1.tile([P, KH, NQ * P], FP16, name="whh")
        nc.sync.dma_start(whh[:], w_hhT_s.rearrange("(k p) c -> p k c", p=P))
        wih = pool1.tile([P, KE, NQ * P], FP16, name="wih")
        nc.sync.dma_start(wih[:], w_ihT_s.rearrange("(k p) c -> p k c", p=P))
        biasT_t = pool1.tile([NQ, P], FP16, name="biasT_t")
        nc.sync.dma_start(biasT_t[:], biasT_s[:])
        onehot_t = pool1.tile([NQ, NQ * B], FP16, name="onehot_t")
        nc.sync.dma_start(onehot_t[:], onehot_s[:])
        bw_t = pool1.tile([P, KH], F32, name="bw_t")
        nc.sync.dma_start(bw_t[:], b_w_sb[:])
        bout_t = pool1.tile([1, VL], FP16, name="bout_t")
        nc.sync.dma_start(bout_t[:], b_out_s[:])
        ones_t = pool1.tile([1, P], FP16, name="ones_t")
        nc.gpsimd.memset(ones_t[:], 1.0)
        h0_t = pool1.tile([P, KH, B], FP16, name="h0_t")
        nc.sync.dma_start(h0_t[:], h0T.rearrange("(k p) b -> p k b", p=P))
        c0_sb = pool1.tile([P, B], F32, name="c0_sb")
        nc.sync.dma_start(c0_sb[:], c0T_s[:])
        ident = pool1.tile([P, P], FP16, name="ident")
        masks.make_identity(nc, ident[:])
        # big weights on the scalar queue so they don't delay the first steps
        ww = pool1.tile([P, 2 * KH, H], FP16, name="ww")
        nc.scalar.dma_start(ww[:], w_wT.rearrange("(k p) m -> p k m", p=P))
        wout = pool1.tile([P, KH, VL], FP16, name="wout")
        nc.scalar.dma_start(wout[:], w_outT_s.rearrange("(k p) v -> p k v", p=P))

        # hall ring: h (fp16) for the last RING steps
        hall = pool1.tile([P, KH, RING, B], FP16, name="hall")

        cc_in = [dram.tile([P, B], FP16, name=f"cc_in{i}") for i in range(T)]
        cc_out = [dram.tile([NCORES * P, B], FP16, name=f"cc_out{i}",
                            addr_space="Shared") for i in range(T)]

        # per-block x_emb tiles, prefetched one block ahead
        xe_tiles = {}

        def fetch_xe(bi):
            t0, t1 = BLOCKS[bi]
            xe = stream.tile([P, KE, B * (t1 - t0)], FP16, name="xe",
                             tag=f"xe{bi % 2}", bufs=1)
            nc.sync.dma_start(xe[:], x_embT_r[:, :, t0 * B:t1 * B])
            xe_tiles[bi] = xe

        # ---------------- filler emission (attention/dec/vocab per block) ----
        # Each closure emits its matmuls and returns a "finisher" (psum
        # consumption: copies/activations/DMAs). The drain runs finishers one
        # quantum later, so a finisher never sits at the head of the DVE/ACT
        # queue waiting on in-flight matmuls and blocking the recurrence's
        # critical elementwise ops (in-order queues = head-of-line blocking).
        def block_closures(bi):
            t0, t1 = BLOCKS[bi]
            bs = t1 - t0
            cols = B * bs            # block cols, b-major (b, t)
            r0 = t0 % RING
            cls = []

            pn_t = work.tile([16, B, S], FP16, name="pn", tag="pn_blk", bufs=1)
            at_t = work.tile([S, B, 16], FP16, name="at", tag="at_blk", bufs=1)
            ctxb = work.tile([P, KH, cols], FP16, name="ctxb", tag="ctx_blk",
                             bufs=1)
            decb = work.tile([P, KH, cols], FP16, name="decb", tag="dec_blk",
                             bufs=1)

            # --- scores (mms) + softmax (finisher), per quad of 4 batches ---
            def mk_scores(q):
                def emit():
                    ps_s = psum.tile([16, 4, S], F32, name="ps_s", tag="ps_sc",
                                     bufs=2)
                    et4 = stream.tile([P, 4, KH, S], FP16, name="et4",
                                      tag="et4", bufs=2)
                    nc.sync.dma_start(
                        et4[:], encT[4 * q:4 * q + 4, :, :].rearrange(
                            "b (k p) s -> p b k s", p=P))
                    for bq in range(4):
                        b = q * 4 + bq
                        for k in range(KH):
                            nc.tensor.matmul(
                                ps_s[:bs, bq, :],
                                lhsT=hall[:, k, r0:r0 + bs, b],
                                rhs=et4[:, bq, k, :],
                                start=(k == 0), stop=(k == KH - 1))

                    def fin():
                        # softmax over s: exp(x) = (1+t)/(1-t), t = tanh(x/2)
                        mxn = work.tile([16, 1], F32, name="mxn", tag="mxn")
                        nc.vector.tensor_reduce(mxn[:bs, :], ps_s[:bs, :, :],
                                                axis=AX.XY, op=ALU.max,
                                                negate=True)
                        nmx2 = work.tile([16, 1], F32, name="nmx2", tag="nmx2")
                        nc.vector.tensor_scalar_mul(nmx2[:bs, :], mxn[:bs, :],
                                                    0.5)
                        tq = work.tile([16, 4, S], F32, name="tq", tag="tq",
                                       bufs=1)
                        nc.scalar.activation(tq[:bs, :, :], ps_s[:bs, :, :],
                                             AF.Tanh, bias=nmx2[:bs, :],
                                             scale=0.5)
                        un = work.tile([16, 4, S], F32, name="un", tag="un",
                                       bufs=1)
                        nc.vector.tensor_scalar_add(un[:bs, :, :],
                                                    tq[:bs, :, :], 1.0)
                        dn = work.tile([16, 4, S], F32, name="dn", tag="dn",
                                       bufs=1)
                        nc.vector.tensor_scalar(dn[:bs, :, :], tq[:bs, :, :],
                                                -1.0, 1.0, ALU.mult, ALU.add)
                        vr = work.tile([16, 4, S], F32, name="vr", tag="vr",
                                       bufs=1)
                        nc.vector.reciprocal(vr[:bs, :, :], dn[:bs, :, :])
                        pu = work.tile([16, 4, S], F32, name="pu", tag="pu",
                                       bufs=1)
                        nc.vector.tensor_tensor(out=pu[:bs, :, :],
                                                in0=un[:bs, :, :],
                                                in1=vr[:bs, :, :], op=ALU.mult)
                        zs = work.tile([16, 4], F32, name="zs", tag="zs")
                        nc.vector.tensor_reduce(zs[:bs, :], pu[:bs, :, :],
                                                axis=AX.X, op=ALU.add)
                        rz = work.tile([16, 4], F32, name="rz", tag="rz")
                        nc.vector.reciprocal(rz[:bs, :], zs[:bs, :])
                        for bq in range(4):
                            b = q * 4 + bq
                            nc.vector.tensor_scalar_mul(
                                pn_t[:bs, b, :], pu[:bs, bq, :],
                                rz[:bs, bq:bq + 1])
                    return fin
                return emit

            # pn_t holds probs [t, b, s]; transpose each b to [s, t]
            def mk_transp(pg):
                def emit():
                    ps_t4 = psum.tile([S, 4, 16], FP16, name="ps_t4",
                                      tag="ps_tr", bufs=1)
                    for i, b in enumerate(range(4 * pg, 4 * pg + 4)):
                        nc.tensor.transpose(
                            ps_t4[:, i, :bs], pn_t[:bs, b, :], ident[:bs, :bs])

                    def fin():
                        for i, b in enumerate(range(4 * pg, 4 * pg + 4)):
                            nc.vector.tensor_copy(at_t[:, b, :bs],
                                                  ps_t4[:, i, :bs])
                    return fin
                return emit

            # context: per k-chunk, all 32 b into one psum bank
            def mk_ctx(k):
                def emit():
                    eca = stream.tile([S, B, P], FP16, name="eca", tag="eca",
                                      bufs=1)
                    nc.sync.dma_start(
                        eca[:], enc[:, :, k * P:(k + 1) * P].rearrange(
                            "b s h -> s b h"))
                    ps_c = psum.tile([P, 512], F32, name="ps_c", tag="ps_ctx",
                                     bufs=1)
                    for b in range(B):
                        nc.tensor.matmul(
                            ps_c[:, b * bs:(b + 1) * bs],
                            lhsT=eca[:, b, :],
                            rhs=at_t[:, b, :bs],
                            start=True, stop=True)

                    def fin():
                        nc.vector.tensor_copy(ctxb[:, k, :], ps_c[:, :cols])
                    return fin
                return emit

            # dec = tanh(W_w^T [h; ctx] + b_w), per output chunk mo
            def mk_dec(mo):
                def emit():
                    ps_d = psum.tile([P, 512], F32, name="ps_d", tag="ps_dec",
                                     bufs=1)
                    for k in range(KH):
                        nc.tensor.matmul(
                            ps_d[:, :cols],
                            lhsT=ww[:, k, mo * P:(mo + 1) * P],
                            rhs=hall[:, k, r0:r0 + bs, :].rearrange(
                                "p t b -> p b t"),
                            start=(k == 0), stop=False)
                    for k in range(KH):
                        nc.tensor.matmul(
                            ps_d[:, :cols],
                            lhsT=ww[:, KH + k, mo * P:(mo + 1) * P],
                            rhs=ctxb[:, k, :],
                            start=False, stop=(k == KH - 1))

                    def fin():
                        nc.scalar.activation(decb[:, mo, :], ps_d[:, :cols],
                                             AF.Tanh, bias=bw_t[:, mo:mo + 1])
                    return fin
                return emit

            # vocab: one closure per (n-tile, 128-col m-window)
            def mk_vocab(n, g):
                def emit():
                    m0 = g * P
                    mw = min(P, cols - m0)
                    ps_v = psum.tile([P, VT], F32, name="ps_v", tag="ps_v")
                    for k in range(KH):
                        nc.tensor.matmul(
                            ps_v[:mw, :],
                            lhsT=decb[:, k, m0:m0 + mw],
                            rhs=wout[:, k, n * VT:(n + 1) * VT],
                            start=(k == 0), stop=(k == KH - 1))

                    def fin():
                        o_sb = work.tile([P, VT], FP16, name="o_sb",
                                         tag="o_sb")
                        nc.vector.tensor_copy(o_sb[:mw, :], ps_v[:mw, :])
                        for (b0, toff, ttake, rel, nb) in _dma_segments(
                                m0, mw, bs):
                            nc.scalar.dma_start(
                                out_s[b0:b0 + nb, t0 + toff:t0 + toff + ttake,
                                      n * VT:(n + 1) * VT],
                                o_sb[rel:rel + nb * ttake, :])
                    return fin
                return emit

            for q in range(8):
                cls.append(mk_scores(q))
            for pg in range(8):
                cls.append(mk_transp(pg))
            for k in range(KH):
                cls.append(mk_ctx(k))
            for mo in range(KH):
                cls.append(mk_dec(mo))
            for n in range(VN):
                for g in range(-(-cols // P)):
                    cls.append(mk_vocab(n, g))
            return cls

        # ---------------- recurrence with interleaved filler ----------------
        fetch_xe(0)
        fetch_xe(1)
        pending = []
        fin_q = []

        def drain_one():
            if fin_q:
                fin_q.pop(0)()
            if pending:
                f = pending.pop(0)()
                if f is not None:
                    fin_q.append(f)

        blocks_done = 0
        cur_blk = 0
        c_prev = c0_sb
        for t in range(T):
            if t >= BLOCKS[cur_blk][1]:
                cur_blk += 1
                if cur_blk + 1 < len(BLOCKS):
                    fetch_xe(cur_blk + 1)
            t0b = BLOCKS[cur_blk][0]
            xe = xe_tiles[cur_blk]
            rt = t % RING
            # gates psum [p, (c, b)], c in (i, f, o, g)
            # NB: start=True zeroes the whole 2KB psum bank (pending-zero is
            # bank-granular), so exactly ONE start per step (the bias matmul).
            psg = psum.tile([P, NQ, B], F32, name="psg", tag="psg", bufs=1)
            # bias + W_ih x_t: h-independent, runs inside the previous AG gap
            for c in range(NQ):
                nc.tensor.matmul(psg[:, c, :], lhsT=biasT_t[:],
                                 rhs=onehot_t[:, c * B:(c + 1) * B],
                                 start=(c == 0), stop=False)
                for k in range(KE):
                    nc.tensor.matmul(
                        psg[:, c, :],
                        lhsT=wih[:, k, c * P:(c + 1) * P],
                        rhs=xe[:, k, (t - t0b) * B:(t - t0b + 1) * B],
                        start=False, stop=False)
            # W_hh h_{t-1}: k-outer so chunks start as their hall DMA lands
            for k in range(KH):
                for c in range(NQ):
                    rhs = (h0_t[:, k, :] if t == 0
                           else hall[:, k, (t - 1) % RING, :])
                    nc.tensor.matmul(
                        psg[:, c, :],
                        lhsT=whh[:, k, c * P:(c + 1) * P],
                        rhs=rhs,
                        start=False,
                        stop=(c == NQ - 1 and k == KH - 1))
            # LSTM elementwise, reading gates straight from PSUM
            sfo = work.tile([P, 3, B], F32, name="sfo", tag="sfo")
            nc.scalar.activation(sfo[:], psg[:, 0:3, :], AF.Sigmoid)
            tg = work.tile([P, B], F32, name="tg", tag="tg")
            nc.scalar.activation(tg[:], psg[:, 3, :], AF.Tanh)
            t1_ = work.tile([P, B], F32, name="t1_", tag="t1_")
            nc.vector.tensor_tensor(out=t1_[:], in0=sfo[:, 1, :],
                                    in1=c_prev[:], op=ALU.mult)
            t2_ = work.tile([P, B], F32, name="t2_", tag="t2_")
            nc.vector.tensor_tensor(out=t2_[:], in0=sfo[:, 0, :], in1=tg[:],
                                    op=ALU.mult)
            c_new = state.tile([P, B], F32, name="c_new", tag="c_new")
            nc.vector.tensor_tensor(out=c_new[:], in0=t1_[:], in1=t2_[:],
                                    op=ALU.add)
            c_prev = c_new
            tc_t = work.tile([P, B], F32, name="tc_t", tag="tc_t")
            nc.scalar.activation(tc_t[:], c_new[:], AF.Tanh)
            h_f = work.tile([P, B], FP16, name="h_f", tag="h_f")
            nc.vector.tensor_tensor(out=h_f[:], in0=sfo[:, 2, :],
                                    in1=tc_t[:], op=ALU.mult)
            # exchange h: SBUF -> DRAM -> AllGather -> hall ring (3 queues)
            nc.gpsimd.dma_start(cc_in[t][:], h_f[:])
            nc.gpsimd.collective_compute(
                "AllGather", ALU.bypass,
                replica_groups=[list(range(NCORES))],
                ins=[cc_in[t].opt()],
                outs=[cc_out[t].opt()])
            cc_r = cc_out[t].rearrange("(k p) b -> p k b", p=P)
            nc.sync.dma_start(hall[:, 0:3, rt, :], cc_r[:, 0:3, :])
            nc.scalar.dma_start(hall[:, 3:6, rt, :], cc_r[:, 3:6, :])
            nc.gpsimd.dma_start(hall[:, 6:8, rt, :], cc_r[:, 6:8, :])
            # drain filler into this step's AG gap
            if pending or fin_q:
                nxt = BLOCKS[blocks_done][1] if blocks_done < len(BLOCKS) else T
                quota = -(-len(pending) // max(1, (3 * (nxt - t)) // 4))
                for _ in range(max(quota, 1)):
                    drain_one()
            if blocks_done < len(BLOCKS) and t + 1 == BLOCKS[blocks_done][1]:
                pending.extend(block_closures(blocks_done))
                blocks_done += 1
        while pending or fin_q:
            drain_one()
    nc.compile()
    return nc


_CACHE = {}


def _get_graph():
    if "nc" not in _CACHE:
        _CACHE["nc"] = build_graph()
    return _CACHE["nc"]


def _prep(tgt_input, hidden_state, cell_state, encoder_outputs,
          embedding, W_ih, W_hh, b_ih, b_hh, W_w, b_w, W_out, b_out):
    """Host-side layout prep. Returns per-core input maps."""
    f32 = np.float32
    bf16 = np.float16
    idx = np.asarray(tgt_input)[:, :-1].astype(np.int64)    # [B, T]
    emb = np.asarray(embedding, f32)[idx]                   # [B, T, E]
    x_embT = np.ascontiguousarray(
        emb.transpose(2, 1, 0).reshape(E, R)).astype(bf16)

    w_ihT = np.asarray(W_ih, f32).T                         # [E, 4H]
    w_hhT = np.asarray(W_hh, f32).T                         # [H, 4H]
    bias = (np.asarray(b_ih, f32) + np.asarray(b_hh, f32))  # [4H]
    h0T_a = np.ascontiguousarray(
        np.asarray(hidden_state, f32)[0].T).astype(bf16)    # [H, B]
    c0T = np.ascontiguousarray(np.asarray(cell_state, f32)[0].T)  # [H, B]
    enc_b = np.asarray(encoder_outputs, f32).astype(bf16)   # [B, S, H]
    encT_b = np.ascontiguousarray(
        np.asarray(encoder_outputs, f32).transpose(0, 2, 1)).astype(bf16)
    w_wT_full = np.ascontiguousarray(np.asarray(W_w, f32).T).astype(bf16)
    b_w_sb = np.ascontiguousarray(np.asarray(b_w, f32).reshape(KH, P).T)
    w_outT = np.asarray(W_out, f32).T                       # [H, V]
    b_out_a = np.asarray(b_out, f32)
    onehot = np.repeat(np.eye(NQ, dtype=f32), B, axis=1).astype(bf16)

    in_maps = []
    for m in range(NCORES):
        # owned gate cols, chunk order (i, f, o, g); PyTorch gate order
        # along 4H is (i, f, g, o) -> quarters (0, 1, 3, 2)
        cols = np.concatenate([np.arange(q * H + m * P, q * H + (m + 1) * P)
                               for q in (0, 1, 3, 2)])
        in_maps.append({
            "x_embT": x_embT,
            "w_ihT_s": np.ascontiguousarray(w_ihT[:, cols]).astype(bf16),
            "w_hhT_s": np.ascontiguousarray(w_hhT[:, cols]).astype(bf16),
            "biasT_s": np.ascontiguousarray(
                bias[cols].reshape(NQ, P)).astype(bf16),
            "onehot_s": onehot,
            "h0T": h0T_a,
            "c0T_s": np.ascontiguousarray(c0T[m * P:(m + 1) * P, :]),
            "enc": enc_b,
            "encT": encT_b,
            "w_wT": w_wT_full,
            "b_w_sb": b_w_sb,
            "w_outT_s": np.ascontiguousarray(
                w_outT[:, m * VL:(m + 1) * VL]).astype(bf16),
            "b_out_s": np.ascontiguousarray(
                b_out_a[m * VL:(m + 1) * VL]).reshape(1, VL).astype(bf16),
        })
    return in_maps


def kernel(**inputs) -> np.ndarray:
    nc = _get_graph()
    in_maps = _prep(**inputs)
    res = run_bass_kernel_spmd(nc, in_maps, list(range(NCORES)))
    outs = [np.asarray(res.results[m]["out_s"], dtype=np.float32)
            for m in range(NCORES)]
    full = np.concatenate(outs, axis=2)
    b_out = np.asarray(inputs["b_out"], np.float32)
    if np.any(b_out):
        full = full + b_out
    return full
